# revision 15
# baseline (speedup 1.0000x reference)
"""Binarized dense layer (tanh(sign(x) @ sign(w) + b)) on 8 Trainium2 cores.

Full input shapes (hardcoded): inputs [8192, 4096] f32, kernel [4096, 4096] f32,
bias [4096] f32 -> out [8192, 4096] f32.

Sharding: 2 batch shards x 4 output-column shards (core i -> r=i//4, c=i%4).
Per core: x [4096, 4096], w [4096, 1024], b [1024] -> y [4096, 1024].

Inputs are shipped to the device as bf16 (sign-preserving for all normal
floats; the binarize only consumes the sign, computed on device), halving
input DMA. All reference ops (binarize X, binarize W, matmul, bias, tanh)
run on device.

Per-core kernel (Tile framework):
  - W: DMA bf16 row-chunks, binarize to +-1 fp8e4 on ACT (Sign), resident
    in SBUF as [128, 2, O] per 256-row K-pair for fp8 DoubleRow matmul.
  - X: DMA bf16 m-tiles [128, 4096]; transpose 128x128 blocks via DMA
    xbar-transpose (SBUF->SBUF, 2-byte dtype); binarize to +-0.5 fp8e4 on
    DVE (tensor_scalar is_ge 0.0 then subtract 0.5) -> lhsT tiles.
  - Matmul: fp8 DoubleRow, K=256 per step, N=512 (one PSUM bank), M=128.
    PSUM accumulates 0.5 * (+-1 dot) exactly in f32. 2 PSUM banks per
    m-tile -> up to 4 m-tiles pipelined.
  - Output: tanh on ACT reading PSUM with scale=2.0 (exact: psum=S/2),
    f32 staged in SBUF, DMA out.
"""

import sys
import types

if "/opt/trn_rl_repo" not in sys.path:
    sys.path.insert(0, "/opt/trn_rl_repo")

from contextlib import ExitStack

import numpy as np
import ml_dtypes

import concourse.bass as bass
import concourse.tile as tile
from concourse import bacc, mybir


def _ensure_ntff_hook_module():
    """The RL image's antenv lacks axon_hooks, which bass_utils imports for
    trace=True under axon. Register a functional shim in sys.modules."""
    name = "antenv.axon_hooks"
    if name in sys.modules:
        return
    try:
        import antenv
        __import__(name)
        return  # real module exists
    except ImportError:
        pass
    mod = types.ModuleType(name)
    mod._hook = None

    def set_axon_ntff_profile_hook(hook):
        mod._hook = hook

    def get_axon_ntff_profile_hook():
        if mod._hook is None:
            try:
                from trn_agent_boot.trn_boot import _ntff_profile_via_ctypes
                mod._hook = _ntff_profile_via_ctypes("/opt/axon/libaxon_pjrt.so")
            except Exception:
                return None
        return mod._hook

    mod.set_axon_ntff_profile_hook = set_axon_ntff_profile_hook
    mod.get_axon_ntff_profile_hook = get_axon_ntff_profile_hook
    sys.modules[name] = mod
    try:
        import antenv
        antenv.axon_hooks = mod
    except ImportError:
        pass


_ensure_ntff_hook_module()

from concourse.bass_utils import run_bass_kernel_spmd  # noqa: E402

F32 = mybir.dt.float32
BF16 = mybir.dt.bfloat16
FP8 = mybir.dt.float8e4

N_CORES = 8
R_SHARDS = 4  # batch shards
C_SHARDS = 2  # output-column shards

B_FULL, D_FULL, O_FULL = 8192, 4096, 4096
B_LOC = B_FULL // R_SHARDS   # 2048
O_LOC = O_FULL // C_SHARDS   # 2048
F16 = mybir.dt.float16


def build_nc(b_loc=B_LOC, d=D_FULL, o_loc=O_LOC, bias_nonzero=False,
             m_split=5, t_lookahead=5):
    """Build the per-core Bass program (identical across cores).

    The first `m_split` m-tiles run their K accumulation in two halves:
    half 1 (k < KP/2) runs while the second half of W is still streaming
    from HBM, with the partial sum evicted to SBUF as fp16 (exact: the
    partial is n/2 with |n/2| <= 1024, representable in fp16), then
    half 2 accumulates in PSUM and the partial is added back. This keeps
    the PE busy during the W-preload phase.
    """
    assert b_loc % 128 == 0 and d % 256 == 0 and o_loc % 512 == 0
    M = b_loc // 128    # m-tiles
    KP = d // 256       # DoubleRow K-pairs
    KC = d // 128       # 128-row chunks of the contraction dim
    N = o_loc // 512    # n-tiles (one PSUM bank each)
    m_split = min(m_split, M)
    if KP < 2:
        m_split = 0
    KH = KP // 2

    nc = bacc.Bacc("TRN2", target_bir_lowering=False, debug=False,
                   num_devices=N_CORES)
    x = nc.dram_tensor("x", [b_loc, d], BF16, kind="ExternalInput")
    w = nc.dram_tensor("w", [d, o_loc], BF16, kind="ExternalInput")
    b = nc.dram_tensor("b", [o_loc], F32, kind="ExternalInput")
    y = nc.dram_tensor("y", [b_loc, o_loc], F32, kind="ExternalOutput")

    with tile.TileContext(nc) as tc, ExitStack() as ctx:
        singles = ctx.enter_context(tc.tile_pool(name="singles", bufs=1))
        wstage = ctx.enter_context(tc.tile_pool(name="wstage", bufs=3))
        wbp = ctx.enter_context(tc.tile_pool(name="wb", bufs=KP))
        xstage = ctx.enter_context(tc.tile_pool(name="xs", bufs=3))
        xbtp = ctx.enter_context(tc.tile_pool(name="xbt",
                                              bufs=m_split + t_lookahead + 2))
        ostage = ctx.enter_context(tc.tile_pool(name="ost", bufs=2))
        partp = ctx.enter_context(tc.tile_pool(name="part", bufs=max(m_split, 1)))
        pstage = ctx.enter_context(tc.tile_pool(name="pt", bufs=2, space="PSUM"))
        pacc = ctx.enter_context(tc.tile_pool(name="pa", bufs=6, space="PSUM"))

        ident = singles.tile([128, 128], BF16)
        from concourse.masks import make_identity
        make_identity(nc, ident)

        bias_bc = None
        if bias_nonzero:
            bias_bc = singles.tile([128, o_loc], F32)
            bias_ap = bass.AP(tensor=b.ap().tensor, offset=0,
                              ap=[[0, 128], [1, o_loc]])
            nc.gpsimd.dma_start(out=bias_bc[:], in_=bias_ap)
            # psum holds S/2 and tanh applies scale=2.0, so add bias/2
            nc.vector.tensor_scalar_mul(bias_bc[:], bias_bc[:], 0.5)

        # first X tiles get a head start on the sync queue before W;
        # the very first is split into quarters so transposes start sooner
        xs_head = {}
        for m in range(min(2, M)):
            xs = xstage.tile([128, d], BF16, tag="xs", name=f"xs{m}")
            if m == 0:
                for qq in range(4):
                    nc.sync.dma_start(
                        out=xs[:, qq * (d // 4):(qq + 1) * (d // 4)],
                        in_=x[0:128, qq * (d // 4):(qq + 1) * (d // 4)])
            else:
                nc.sync.dma_start(out=xs[:], in_=x[m * 128:(m + 1) * 128, :])
            xs_head[m] = xs

        # ---- W preload: binarize to +-1 fp8, resident in SBUF ----
        # wb[k] is [128, 2, o_loc]: (p, j, n) = sign(w[k*256 + j*128 + p, n])
        wb = []
        for k in range(KP):
            t = wbp.tile([128, 2, o_loc], FP8, tag="wb", name=f"wb{k}")
            for j in (0, 1):
                s = wstage.tile([128, o_loc], BF16, tag="ws", name=f"ws{k}_{j}")
                nc.sync.dma_start(
                    out=s[:], in_=w[(2 * k + j) * 128:(2 * k + j + 1) * 128, :])
                nc.scalar.activation(out=t[:, j, :], in_=s[:],
                                     func=mybir.ActivationFunctionType.Sign)
            wb.append(t)

        def load_and_transpose(m):
            if m in xs_head:
                xs = xs_head[m]
            else:
                xs = xstage.tile([128, d], BF16, tag="xs", name=f"xs{m}")
                nc.gpsimd.dma_start(out=xs[:], in_=x[m * 128:(m + 1) * 128, :])
            # PE-transpose 128x128 bf16 blocks into PSUM (4 per bank), then
            # binarize+evict on DVE: xbt[p, c, mm] = +-0.5 of x[m*128+mm, c*128+p]
            xbt = xbtp.tile([128, KC, 128], FP8, tag="xbt", name=f"xbt{m}")
            for q in range(KC // 4):
                pt = pstage.tile([128, 4, 128], BF16, tag="pt", name=f"pt{m}_{q}")
                for i in range(4):
                    c = 4 * q + i
                    nc.tensor.transpose(pt[:, i, :],
                                        xs[:, c * 128:(c + 1) * 128], ident[:])
                nc.vector.tensor_scalar(
                    out=xbt[:, 4 * q:4 * q + 4, :], in0=pt[:],
                    scalar1=0.0, scalar2=0.5,
                    op0=mybir.AluOpType.is_ge, op1=mybir.AluOpType.subtract)
            return xbt

        def k_group(pa, xbt, k0, k1, first):
            for k in range(k0, k1):
                lhsT = xbt[:, 2 * k:2 * k + 2, :]
                for n in range(N):
                    nc.tensor.matmul(
                        pa[n][:], lhsT, wb[k][:, :, n * 512:(n + 1) * 512],
                        start=(k == k0 and first), stop=(k == k1 - 1),
                        perf_mode=mybir.MatmulPerfMode.DoubleRow)

        def finish(m, pa):
            o = ostage.tile([128, o_loc], F32, tag="o", name=f"o{m}")
            for n in range(N):
                pn = pa[n][:]
                if bias_bc is not None:
                    nc.vector.tensor_tensor(
                        out=pn, in0=pn, in1=bias_bc[:, n * 512:(n + 1) * 512],
                        op=mybir.AluOpType.add)
                nc.scalar.activation(out=o[:, n * 512:(n + 1) * 512], in_=pn,
                                     func=mybir.ActivationFunctionType.Tanh,
                                     scale=2.0)
            nc.sync.dma_start(out=y[m * 128:(m + 1) * 128, :], in_=o[:])

        def alloc_pa(m):
            return [pacc.tile([128, 512], F32, tag="pa", name=f"pa_{m}_{n}")
                    for n in range(N)]

        # ---- phase 1: first m_split tiles accumulate k < KH while the
        # second half of W streams in; partials spill to SBUF as fp16 ----
        xbts = {}
        parts = {}
        for m in range(m_split):
            xbts[m] = load_and_transpose(m)
            pa = alloc_pa(m)
            k_group(pa, xbts[m], 0, KH, first=True)
            part = partp.tile([128, N, 512], F16, tag="part", name=f"part{m}")
            for n in range(N):
                nc.vector.tensor_copy(out=part[:, n, :], in_=pa[n][:])
            parts[m] = part

        # ---- drain: interleave remaining transposes (lookahead) with the
        # phase-2 finishes and the remaining single-pass k loops, so the
        # PE has transpose work while the tail of W streams in ----
        todo_T = list(range(m_split, M))
        consumers = [("p2", m) for m in range(m_split)] + \
                    [("full", m) for m in range(m_split, M)]
        emitted_T = set(range(m_split))

        def emit_T():
            if todo_T:
                mt = todo_T.pop(0)
                xbts[mt] = load_and_transpose(mt)
                emitted_T.add(mt)

        for kind, m in consumers:
            if kind == "full":
                while m not in emitted_T:
                    emit_T()
            emit_T()
            pa = alloc_pa(m)
            if kind == "p2":
                k_group(pa, xbts[m], KH, KP, first=True)
                for n in range(N):
                    nc.vector.tensor_tensor(out=pa[n][:], in0=pa[n][:],
                                            in1=parts[m][:, n, :],
                                            op=mybir.AluOpType.add)
            else:
                k_group(pa, xbts[m], 0, KP, first=True)
            finish(m, pa)

    nc.compile()
    return nc


_NC_CACHE = {}


def _get_nc(key, **kwargs):
    if key not in _NC_CACHE:
        _NC_CACHE[key] = build_nc(**kwargs)
    return _NC_CACHE[key]


def kernel(inputs: np.ndarray, kernel: np.ndarray, bias: np.ndarray,
           _trace: bool = False, _trace_cores=None) -> np.ndarray:
    x = np.asarray(inputs, dtype=np.float32).astype(ml_dtypes.bfloat16)
    w = np.asarray(kernel, dtype=np.float32).astype(ml_dtypes.bfloat16)
    b = np.ascontiguousarray(bias, dtype=np.float32)
    assert x.shape == (B_FULL, D_FULL) and w.shape == (D_FULL, O_FULL)

    bias_nonzero = bool(np.any(b != 0))
    nc = _get_nc(("full", bias_nonzero), bias_nonzero=bias_nonzero)

    in_maps = []
    for i in range(N_CORES):
        r, c = i // C_SHARDS, i % C_SHARDS
        in_maps.append({
            "x": np.ascontiguousarray(x[r * B_LOC:(r + 1) * B_LOC, :]),
            "w": np.ascontiguousarray(w[:, c * O_LOC:(c + 1) * O_LOC]),
            "b": np.ascontiguousarray(b[c * O_LOC:(c + 1) * O_LOC]),
        })

    res = run_bass_kernel_spmd(nc, in_maps, list(range(N_CORES)),
                               trace=_trace, trace_cores=_trace_cores)

    out = np.empty((B_FULL, O_FULL), dtype=np.float32)
    for i in range(N_CORES):
        r, c = i // C_SHARDS, i % C_SHARDS
        out[r * B_LOC:(r + 1) * B_LOC, c * O_LOC:(c + 1) * O_LOC] = \
            res.results[i]["y"]

    if _trace:
        return out, res
    return out


# revision 16
# speedup vs baseline: 1.0144x; 1.0144x over previous
"""Binarized dense layer (tanh(sign(x) @ sign(w) + b)) on 8 Trainium2 cores.

Full input shapes (hardcoded): inputs [8192, 4096] f32, kernel [4096, 4096] f32,
bias [4096] f32 -> out [8192, 4096] f32.

Sharding: 4 batch shards x 2 output-column shards (core i -> r=i//2, c=i%2).
Per core: x [2048, 4096], w [4096, 2048], b [2048] -> y [2048, 2048].

Wire format: inputs are shipped as bf16 (sign-preserving for all normal
floats; the binarize consumes only the sign, computed on device), and the
X shard is laid out K-major (transposed) so the contraction dim lands on
SBUF partitions directly. All reference ops (binarize X, binarize W,
matmul, bias add, tanh) run on device.

Per-core kernel (Tile framework):
  - W: DMA bf16 row-chunks, binarize to +-1 fp8e4 on ACT (Sign), resident
    in SBUF as [128, 2, O] per 256-row K-pair for fp8 DoubleRow matmul.
  - X^T: DMA bf16 k-chunks [128, B], binarize to +-0.5 fp8e4 with
    tensor_scalar (is_ge 0.0, subtract 0.5) on DVE/GpSimd; the whole
    binarized X^T (8.4 MB fp8) stays resident in SBUF.
  - Matmul: fp8 DoubleRow, K=256 per step, N=512 (one PSUM bank), M=128.
    PSUM accumulates 0.5 * (+-1 dot) exactly in f32.
  - Streaming phase: the first m_split m-tiles run k < KP/2 while the
    second halves of W/X^T stream in; partials spill to SBUF as fp16
    (exact: values are n/2 with |n/2| <= 1024) and are added back later.
  - Output: tanh on ACT reading PSUM with scale=2.0 (exact: psum = S/2),
    f32 staged in SBUF, DMA out.
"""

import sys
import types

if "/opt/trn_rl_repo" not in sys.path:
    sys.path.insert(0, "/opt/trn_rl_repo")

from contextlib import ExitStack

import numpy as np
import ml_dtypes

import concourse.bass as bass
import concourse.tile as tile
from concourse import bacc, mybir


def _ensure_ntff_hook_module():
    """The RL image's antenv lacks axon_hooks, which bass_utils imports for
    trace=True under axon. Register a functional shim in sys.modules."""
    name = "antenv.axon_hooks"
    if name in sys.modules:
        return
    try:
        import antenv
        __import__(name)
        return  # real module exists
    except ImportError:
        pass
    mod = types.ModuleType(name)
    mod._hook = None

    def set_axon_ntff_profile_hook(hook):
        mod._hook = hook

    def get_axon_ntff_profile_hook():
        if mod._hook is None:
            try:
                from trn_agent_boot.trn_boot import _ntff_profile_via_ctypes
                mod._hook = _ntff_profile_via_ctypes("/opt/axon/libaxon_pjrt.so")
            except Exception:
                return None
        return mod._hook

    mod.set_axon_ntff_profile_hook = set_axon_ntff_profile_hook
    mod.get_axon_ntff_profile_hook = get_axon_ntff_profile_hook
    sys.modules[name] = mod
    try:
        import antenv
        antenv.axon_hooks = mod
    except ImportError:
        pass


_ensure_ntff_hook_module()

from concourse.bass_utils import run_bass_kernel_spmd  # noqa: E402

F32 = mybir.dt.float32
F16 = mybir.dt.float16
BF16 = mybir.dt.bfloat16
FP8 = mybir.dt.float8e4

N_CORES = 8
R_SHARDS = 4  # batch shards
C_SHARDS = 2  # output-column shards

B_FULL, D_FULL, O_FULL = 8192, 4096, 4096
B_LOC = B_FULL // R_SHARDS   # 2048
O_LOC = O_FULL // C_SHARDS   # 2048


def build_nc(b_loc=B_LOC, d=D_FULL, o_loc=O_LOC, bias_nonzero=False,
             m_split=6):
    """Build the per-core Bass program (identical across cores)."""
    assert b_loc % 128 == 0 and d % 256 == 0 and o_loc % 512 == 0
    M = b_loc // 128    # m-tiles
    KP = d // 256       # DoubleRow K-pairs
    KC = d // 128       # 128-row chunks of the contraction dim
    N = o_loc // 512    # n-tiles (one PSUM bank each)
    m_split = min(m_split, M)
    if KP < 2:
        m_split = 0
    KH = KP // 2
    KCH = KC // 2

    nc = bacc.Bacc("TRN2", target_bir_lowering=False, debug=False,
                   num_devices=N_CORES)
    # x is the TRANSPOSED shard: [d, b_loc], K-major
    x = nc.dram_tensor("x", [d, b_loc], BF16, kind="ExternalInput")
    w = nc.dram_tensor("w", [d, o_loc], BF16, kind="ExternalInput")
    b = nc.dram_tensor("b", [o_loc], F32, kind="ExternalInput")
    y = nc.dram_tensor("y", [b_loc, o_loc], F32, kind="ExternalOutput")

    with tile.TileContext(nc) as tc, ExitStack() as ctx:
        singles = ctx.enter_context(tc.tile_pool(name="singles", bufs=1))
        wstage = ctx.enter_context(tc.tile_pool(name="wstage", bufs=3))
        wbp = ctx.enter_context(tc.tile_pool(name="wb", bufs=KP))
        xstage = ctx.enter_context(tc.tile_pool(name="xs", bufs=3))
        xbtp = ctx.enter_context(tc.tile_pool(name="xbt", bufs=1))
        ostage = ctx.enter_context(tc.tile_pool(name="ost", bufs=6))
        partp = ctx.enter_context(tc.tile_pool(name="part",
                                               bufs=max(m_split, 1)))
        pacc = ctx.enter_context(tc.tile_pool(name="pa", bufs=8,
                                              space="PSUM"))

        bias_bc = None
        if bias_nonzero:
            bias_bc = singles.tile([128, o_loc], F32)
            bias_ap = bass.AP(tensor=b.ap().tensor, offset=0,
                              ap=[[0, 128], [1, o_loc]])
            nc.gpsimd.dma_start(out=bias_bc[:], in_=bias_ap)
            # psum holds S/2 and tanh applies scale=2.0, so add bias/2
            nc.vector.tensor_scalar_mul(bias_bc[:], bias_bc[:], 0.5)

        # whole binarized X^T stays resident: [128, KC, b_loc] fp8
        xbt = xbtp.tile([128, KC, b_loc], FP8)

        def load_x_chunk(c, engine):
            xs = xstage.tile([128, b_loc], BF16, tag="xs", name=f"xs{c}")
            nc.gpsimd.dma_start(out=xs[:], in_=x[c * 128:(c + 1) * 128, :])
            # binarize to +-0.5 fp8 in one pass
            engine.tensor_scalar(
                out=xbt[:, c, :], in0=xs[:], scalar1=0.0, scalar2=0.5,
                op0=mybir.AluOpType.is_ge, op1=mybir.AluOpType.subtract)

        def load_w_pair(k):
            t = wbp.tile([128, 2, o_loc], FP8, tag="wb", name=f"wb{k}")
            for j in (0, 1):
                s = wstage.tile([128, o_loc], BF16, tag="ws", name=f"ws{k}_{j}")
                nc.sync.dma_start(
                    out=s[:], in_=w[(2 * k + j) * 128:(2 * k + j + 1) * 128, :])
                nc.scalar.activation(out=t[:, j, :], in_=s[:],
                                     func=mybir.ActivationFunctionType.Sign)
            return t

        # ---- first halves of X^T and W (X on gpsimd queue, W on sync) ----
        for c in range(KCH):
            load_x_chunk(c, nc.vector)
        wb = []
        for k in range(KH):
            wb.append(load_w_pair(k))

        def k_group(pa, m, k0, k1):
            for k in range(k0, k1):
                lhsT = xbt[:, 2 * k:2 * k + 2, m * 128:(m + 1) * 128]
                for n in range(N):
                    nc.tensor.matmul(
                        pa[n][:], lhsT, wb[k][:, :, n * 512:(n + 1) * 512],
                        start=(k == k0), stop=(k == k1 - 1),
                        perf_mode=mybir.MatmulPerfMode.DoubleRow)

        def finish(m, pa):
            o = ostage.tile([128, o_loc // 2], F32, tag="o", name=f"o{m}")
            for half in range(2):
                for nn in range(N // 2):
                    n = half * (N // 2) + nn
                    pn = pa[n][:]
                    if bias_bc is not None:
                        nc.vector.tensor_tensor(
                            out=pn, in0=pn,
                            in1=bias_bc[:, n * 512:(n + 1) * 512],
                            op=mybir.AluOpType.add)
                    nc.scalar.activation(
                        out=o[:, nn * 512:(nn + 1) * 512], in_=pn,
                        func=mybir.ActivationFunctionType.Tanh, scale=2.0)
                nc.sync.dma_start(
                    out=y[m * 128:(m + 1) * 128,
                          half * (o_loc // 2):(half + 1) * (o_loc // 2)],
                    in_=o[:])
                if half == 0:
                    o = ostage.tile([128, o_loc // 2], F32, tag="o",
                                    name=f"o{m}b")

        def alloc_pa(m):
            return [pacc.tile([128, 512], F32, tag="pa", name=f"pa_{m}_{n}")
                    for n in range(N)]

        # ---- phase 1: first m_split tiles accumulate k < KH while the
        # second halves stream in; partials spill to SBUF as fp16; the
        # second-half X binarizes are interleaved on the DVE queue so they
        # don't block behind (or get blocked by) the partial evictions ----
        parts = {}
        c2 = KCH  # next second-half X chunk to emit
        k2 = KH   # next second-half W pair to emit
        for m in range(m_split):
            pa = alloc_pa(m)
            k_group(pa, m, 0, KH)
            part = partp.tile([128, N, 512], F16, tag="part", name=f"part{m}")
            for n in range(N):
                nc.vector.tensor_copy(out=part[:, n, :], in_=pa[n][:])
            parts[m] = part
            # interleave a slice of the second-half loads
            for _ in range((KCH + m_split - 1) // m_split):
                if c2 < KC:
                    load_x_chunk(c2, nc.vector)
                    c2 += 1
            for _ in range((KH + m_split - 1) // m_split):
                if k2 < KP:
                    wb.append(load_w_pair(k2))
                    k2 += 1
        while c2 < KC:
            load_x_chunk(c2, nc.vector)
            c2 += 1
        while k2 < KP:
            wb.append(load_w_pair(k2))
            k2 += 1

        # ---- phase 2: finish the split tiles (k >= KH, add partial) ----
        for m in range(m_split):
            pa = alloc_pa(m)
            k_group(pa, m, KH, KP)
            for n in range(N):
                nc.vector.tensor_tensor(out=pa[n][:], in0=pa[n][:],
                                        in1=parts[m][:, n, :],
                                        op=mybir.AluOpType.add)
            finish(m, pa)

        # ---- remaining m-tiles: single-pass k loop ----
        for m in range(m_split, M):
            pa = alloc_pa(m)
            k_group(pa, m, 0, KP)
            finish(m, pa)

    nc.compile()
    return nc


_NC_CACHE = {}


def _get_nc(key, **kwargs):
    if key not in _NC_CACHE:
        _NC_CACHE[key] = build_nc(**kwargs)
    return _NC_CACHE[key]


def kernel(inputs: np.ndarray, kernel: np.ndarray, bias: np.ndarray,
           _trace: bool = False, _trace_cores=None) -> np.ndarray:
    x = np.asarray(inputs, dtype=np.float32).astype(ml_dtypes.bfloat16)
    w = np.asarray(kernel, dtype=np.float32).astype(ml_dtypes.bfloat16)
    b = np.ascontiguousarray(bias, dtype=np.float32)
    assert x.shape == (B_FULL, D_FULL) and w.shape == (D_FULL, O_FULL)

    bias_nonzero = bool(np.any(b != 0))
    nc = _get_nc(("full", bias_nonzero), bias_nonzero=bias_nonzero)

    in_maps = []
    for i in range(N_CORES):
        r, c = i // C_SHARDS, i % C_SHARDS
        in_maps.append({
            "x": np.ascontiguousarray(x[r * B_LOC:(r + 1) * B_LOC, :].T),
            "w": np.ascontiguousarray(w[:, c * O_LOC:(c + 1) * O_LOC]),
            "b": np.ascontiguousarray(b[c * O_LOC:(c + 1) * O_LOC]),
        })

    res = run_bass_kernel_spmd(nc, in_maps, list(range(N_CORES)),
                               trace=_trace, trace_cores=_trace_cores)

    out = np.empty((B_FULL, O_FULL), dtype=np.float32)
    for i in range(N_CORES):
        r, c = i // C_SHARDS, i % C_SHARDS
        out[r * B_LOC:(r + 1) * B_LOC, c * O_LOC:(c + 1) * O_LOC] = \
            res.results[i]["y"]

    if _trace:
        return out, res
    return out


# revision 21
# speedup vs baseline: 1.0393x; 1.0245x over previous
"""Binarized dense layer (tanh(sign(x) @ sign(w) + b)) on 8 Trainium2 cores.

Full input shapes (hardcoded): inputs [8192, 4096] f32, kernel [4096, 4096] f32,
bias [4096] f32 -> out [8192, 4096] f32.

Sharding: 4 batch shards x 2 output-column shards (core i -> r=i//2, c=i%2).
Per core: x [2048, 4096], w [4096, 2048], b [2048] -> y [2048, 2048].

Wire format: inputs are shipped as bf16 (sign-preserving for all normal
floats; the binarize consumes only the sign, computed on device), and the
X shard is laid out K-major (transposed) so the contraction dim lands on
SBUF partitions directly. All reference ops (binarize X, binarize W,
matmul, bias add, tanh) run on device.

Per-core kernel (Tile framework):
  - W: DMA bf16 row-chunks, binarize to +-1 fp8e4 on ACT (Sign), resident
    in SBUF as [128, 2, O] per 256-row K-pair for fp8 DoubleRow matmul.
  - X^T: DMA bf16 k-chunks [128, B], binarize to +-0.5 fp8e4 with
    tensor_scalar (is_ge 0.0, subtract 0.5) on DVE/GpSimd; the whole
    binarized X^T (8.4 MB fp8) stays resident in SBUF.
  - Matmul: fp8 DoubleRow, K=256 per step, N=512 (one PSUM bank), M=128.
    PSUM accumulates 0.5 * (+-1 dot) exactly in f32.
  - Streaming phase: the first m_split m-tiles run k < KP/2 while the
    second halves of W/X^T stream in; partials spill to SBUF as fp16
    (exact: values are n/2 with |n/2| <= 1024) and are added back later.
  - Output: tanh on ACT reading PSUM with scale=2.0 (exact: psum = S/2),
    f32 staged in SBUF, DMA out.
"""

import sys
import types

if "/opt/trn_rl_repo" not in sys.path:
    sys.path.insert(0, "/opt/trn_rl_repo")

from contextlib import ExitStack

import numpy as np
import ml_dtypes

import concourse.bass as bass
import concourse.tile as tile
from concourse import bacc, mybir


def _ensure_ntff_hook_module():
    """The RL image's antenv lacks axon_hooks, which bass_utils imports for
    trace=True under axon. Register a functional shim in sys.modules."""
    name = "antenv.axon_hooks"
    if name in sys.modules:
        return
    try:
        import antenv
        __import__(name)
        return  # real module exists
    except ImportError:
        pass
    mod = types.ModuleType(name)
    mod._hook = None

    def set_axon_ntff_profile_hook(hook):
        mod._hook = hook

    def get_axon_ntff_profile_hook():
        if mod._hook is None:
            try:
                from trn_agent_boot.trn_boot import _ntff_profile_via_ctypes
                mod._hook = _ntff_profile_via_ctypes("/opt/axon/libaxon_pjrt.so")
            except Exception:
                return None
        return mod._hook

    mod.set_axon_ntff_profile_hook = set_axon_ntff_profile_hook
    mod.get_axon_ntff_profile_hook = get_axon_ntff_profile_hook
    sys.modules[name] = mod
    try:
        import antenv
        antenv.axon_hooks = mod
    except ImportError:
        pass


_ensure_ntff_hook_module()

from concourse.bass_utils import run_bass_kernel_spmd  # noqa: E402

F32 = mybir.dt.float32
F16 = mybir.dt.float16
BF16 = mybir.dt.bfloat16
FP8 = mybir.dt.float8e4

N_CORES = 8
R_SHARDS = 4  # batch shards
C_SHARDS = 2  # output-column shards

B_FULL, D_FULL, O_FULL = 8192, 4096, 4096
B_LOC = B_FULL // R_SHARDS   # 2048
O_LOC = O_FULL // C_SHARDS   # 2048


def build_nc(b_loc=B_LOC, d=D_FULL, o_loc=O_LOC, bias_nonzero=False,
             m_split=10, warmers=True):
    """Build the per-core Bass program (identical across cores)."""
    assert b_loc % 128 == 0 and d % 256 == 0 and o_loc % 512 == 0
    M = b_loc // 128    # m-tiles
    KP = d // 256       # DoubleRow K-pairs
    KC = d // 128       # 128-row chunks of the contraction dim
    N = o_loc // 512    # n-tiles (one PSUM bank each)
    m_split = min(m_split, M)
    if KP < 2:
        m_split = 0
    KH = KP // 2
    KCH = KC // 2

    nc = bacc.Bacc("TRN2", target_bir_lowering=False, debug=False,
                   num_devices=N_CORES)
    # x is the TRANSPOSED shard: [d, b_loc], K-major
    x = nc.dram_tensor("x", [d, b_loc], BF16, kind="ExternalInput")
    w = nc.dram_tensor("w", [d, o_loc], BF16, kind="ExternalInput")
    b = nc.dram_tensor("b", [o_loc], F32, kind="ExternalInput")
    y = nc.dram_tensor("y", [b_loc, o_loc], F32, kind="ExternalOutput")

    with tile.TileContext(nc) as tc, ExitStack() as ctx:
        singles = ctx.enter_context(tc.tile_pool(name="singles", bufs=1))
        wstage = ctx.enter_context(tc.tile_pool(name="wstage", bufs=3))
        wbp = ctx.enter_context(tc.tile_pool(name="wb", bufs=KP))
        xstage = ctx.enter_context(tc.tile_pool(name="xs", bufs=2))
        xbtp = ctx.enter_context(tc.tile_pool(name="xbt", bufs=1))
        ostage = ctx.enter_context(tc.tile_pool(name="ost", bufs=4))
        partp = ctx.enter_context(tc.tile_pool(name="part",
                                               bufs=max(m_split, 1)))
        pacc = ctx.enter_context(tc.tile_pool(name="pa", bufs=7,
                                              space="PSUM"))
        pscr = ctx.enter_context(tc.tile_pool(name="pscr", bufs=1,
                                              space="PSUM"))
        scratch = (pscr.tile([128, 64], F32, name="scratch")
                   if warmers else None)

        bias_bc = None
        if bias_nonzero:
            bias_bc = singles.tile([128, o_loc], F32)
            bias_ap = bass.AP(tensor=b.ap().tensor, offset=0,
                              ap=[[0, 128], [1, o_loc]])
            nc.gpsimd.dma_start(out=bias_bc[:], in_=bias_ap)
            # psum holds S/2 and tanh applies scale=2.0, so add bias/2
            nc.vector.tensor_scalar_mul(bias_bc[:], bias_bc[:], 0.5)

        # whole binarized X^T stays resident: [128, KC, b_loc] fp8
        xbt = xbtp.tile([128, KC, b_loc], FP8)

        def load_x_chunk(c, engine):
            xs = xstage.tile([128, b_loc], BF16, tag="xs", name=f"xs{c}")
            nc.gpsimd.dma_start(out=xs[:], in_=x[c * 128:(c + 1) * 128, :])
            # binarize to +-0.5 fp8 in one pass
            engine.tensor_scalar(
                out=xbt[:, c, :], in0=xs[:], scalar1=0.0, scalar2=0.5,
                op0=mybir.AluOpType.is_ge, op1=mybir.AluOpType.subtract)
            if scratch is not None:
                # tiny matmul paced by this chunk's arrival: keeps the PE HAM
                # activity window busy through the load phase (else the PE
                # re-throttles to 1.2 GHz between sparse real matmuls)
                nc.tensor.matmul(scratch[:], xbt[:, c, 0:128],
                                 xbt[:, c, 0:64], start=True, stop=True)

        def load_w_pair(k):
            t = wbp.tile([128, 2, o_loc], FP8, tag="wb", name=f"wb{k}")
            for j in (0, 1):
                s = wstage.tile([128, o_loc], BF16, tag="ws", name=f"ws{k}_{j}")
                nc.sync.dma_start(
                    out=s[:], in_=w[(2 * k + j) * 128:(2 * k + j + 1) * 128, :])
                nc.scalar.activation(out=t[:, j, :], in_=s[:],
                                     func=mybir.ActivationFunctionType.Sign)
            return t

        # ---- first halves of X^T and W (X on gpsimd queue, W on sync) ----
        for c in range(KCH):
            load_x_chunk(c, nc.vector)
        wb = []
        for k in range(KH):
            wb.append(load_w_pair(k))

        def k_group(pa, m, k0, k1):
            for k in range(k0, k1):
                lhsT = xbt[:, 2 * k:2 * k + 2, m * 128:(m + 1) * 128]
                for n in range(N):
                    nc.tensor.matmul(
                        pa[n][:], lhsT, wb[k][:, :, n * 512:(n + 1) * 512],
                        start=(k == k0), stop=(k == k1 - 1),
                        perf_mode=mybir.MatmulPerfMode.DoubleRow)

        def finish(m, pa):
            o = ostage.tile([128, o_loc // 2], F32, tag="o", name=f"o{m}")
            for half in range(2):
                for nn in range(N // 2):
                    n = half * (N // 2) + nn
                    pn = pa[n][:]
                    if bias_bc is not None:
                        nc.vector.tensor_tensor(
                            out=pn, in0=pn,
                            in1=bias_bc[:, n * 512:(n + 1) * 512],
                            op=mybir.AluOpType.add)
                    nc.scalar.activation(
                        out=o[:, nn * 512:(nn + 1) * 512], in_=pn,
                        func=mybir.ActivationFunctionType.Tanh, scale=2.0)
                nc.sync.dma_start(
                    out=y[m * 128:(m + 1) * 128,
                          half * (o_loc // 2):(half + 1) * (o_loc // 2)],
                    in_=o[:])
                if half == 0:
                    o = ostage.tile([128, o_loc // 2], F32, tag="o",
                                    name=f"o{m}b")

        def alloc_pa(m):
            return [pacc.tile([128, 512], F32, tag="pa", name=f"pa_{m}_{n}")
                    for n in range(N)]

        # ---- phase 1: first m_split tiles accumulate k < KH while the
        # second halves stream in; partials spill to SBUF as fp16; the
        # second-half X binarizes are interleaved on the DVE queue so they
        # don't block behind (or get blocked by) the partial evictions ----
        parts = {}
        c2 = KCH  # next second-half X chunk to emit
        k2 = KH   # next second-half W pair to emit
        for m in range(m_split):
            pa = alloc_pa(m)
            k_group(pa, m, 0, KH)
            part = partp.tile([128, N, 512], F16, tag="part", name=f"part{m}")
            for n in range(N):
                nc.vector.tensor_copy(out=part[:, n, :], in_=pa[n][:])
            parts[m] = part
            # interleave a slice of the second-half loads
            for _ in range((KCH + m_split - 1) // m_split):
                if c2 < KC:
                    load_x_chunk(c2, nc.vector)
                    c2 += 1
            for _ in range((KH + m_split - 1) // m_split):
                if k2 < KP:
                    wb.append(load_w_pair(k2))
                    k2 += 1
        while c2 < KC:
            load_x_chunk(c2, nc.vector)
            c2 += 1
        while k2 < KP:
            wb.append(load_w_pair(k2))
            k2 += 1

        # ---- phase 2: finish the split tiles (k >= KH, add partial) ----
        for m in range(m_split):
            pa = alloc_pa(m)
            k_group(pa, m, KH, KP)
            for n in range(N):
                nc.vector.tensor_tensor(out=pa[n][:], in0=pa[n][:],
                                        in1=parts[m][:, n, :],
                                        op=mybir.AluOpType.add)
            finish(m, pa)

        # ---- remaining m-tiles: single-pass k loop ----
        for m in range(m_split, M):
            pa = alloc_pa(m)
            k_group(pa, m, 0, KP)
            finish(m, pa)

    nc.compile()
    return nc


_NC_CACHE = {}


def _get_nc(key, **kwargs):
    if key not in _NC_CACHE:
        _NC_CACHE[key] = build_nc(**kwargs)
    return _NC_CACHE[key]


def kernel(inputs: np.ndarray, kernel: np.ndarray, bias: np.ndarray,
           _trace: bool = False, _trace_cores=None) -> np.ndarray:
    x = np.asarray(inputs, dtype=np.float32).astype(ml_dtypes.bfloat16)
    w = np.asarray(kernel, dtype=np.float32).astype(ml_dtypes.bfloat16)
    b = np.ascontiguousarray(bias, dtype=np.float32)
    assert x.shape == (B_FULL, D_FULL) and w.shape == (D_FULL, O_FULL)

    bias_nonzero = bool(np.any(b != 0))
    nc = _get_nc(("full", bias_nonzero), bias_nonzero=bias_nonzero,
                 m_split=8 if bias_nonzero else 10)

    in_maps = []
    for i in range(N_CORES):
        r, c = i // C_SHARDS, i % C_SHARDS
        in_maps.append({
            "x": np.ascontiguousarray(x[r * B_LOC:(r + 1) * B_LOC, :].T),
            "w": np.ascontiguousarray(w[:, c * O_LOC:(c + 1) * O_LOC]),
            "b": np.ascontiguousarray(b[c * O_LOC:(c + 1) * O_LOC]),
        })

    res = run_bass_kernel_spmd(nc, in_maps, list(range(N_CORES)),
                               trace=_trace, trace_cores=_trace_cores)

    out = np.empty((B_FULL, O_FULL), dtype=np.float32)
    for i in range(N_CORES):
        r, c = i // C_SHARDS, i % C_SHARDS
        out[r * B_LOC:(r + 1) * B_LOC, c * O_LOC:(c + 1) * O_LOC] = \
            res.results[i]["y"]

    if _trace:
        return out, res
    return out
